# revision 41
# baseline (speedup 1.0000x reference)
# MoE (15 routed experts, top-2, + shared expert) on 8 trn2 NeuronCores.
#
# Strategy: all routing runs on the HOST (fp64 gate -> top-2 -> per-expert
# token lists -> packed dense inputs); the device kernel is a pure static
# dense-FFN pipeline, which keeps the PE streaming with zero serial
# dispatch chain. 24 uniform slots of CAP=512 tokens across 8 cores
# (3 per core): 8 shared-expert slots (data-parallel, 512 tokens each),
# 15 slots holding the first <=512 tokens of each routed expert, and one
# spare slot holding the largest overflow chunk. The remaining overflow
# (~400 expert-tokens, <1% of the work) is computed on the host in numpy
# during the scatter-add. Uniform 512-token slots keep every PSUM block
# exactly 512 wide and every y token-tile exactly 128 -- no ragged
# matmuls, minimum issued columns (512*24 token-slots = the ideal
# 1536/core). Combine weights (top-2 softmax probs) are applied on the
# host during the scatter-add, so the device computes plain SwiGLU FFNs.
#
# Matmul structure per slot: h-phase keeps W1/W3 tiles stationary and
# streams all 512 tokens per weight load; y-phase keeps 128-token h tiles
# stationary and streams the full 2048-wide W2^T per load (2048 columns
# per LDWEIGHTS). Redundant LDWEIGHTS are deduped by rewriting the
# serialized BIR before walrus codegen (_dedup_ldw): the tile scheduler
# emits one Ldweights per matmul even for same-weight runs, and each
# redundant reload would force a PE pipeline drain (~270ns). DMA is
# ordered for overlap: the w1/w3 stream owns the sync DGE ring, while x
# staging, w2^T prefetch, and y writebacks ride the scalar ring; slot0's
# first x/weight tiles are sliced into 4-dk chunks so the first matmul
# fires as early as possible. y is written back in bf16 to halve the
# output DMA (host upcasts during the combine).
import numpy as np
import ml_dtypes

DIM = 2048
INTER = 1408
NE = 15
TOPK = 2
T = 4096
NCORES = 8
CAP = 512             # uniform slot token capacity
DI = DIM // 128       # 16 contraction tiles over d
II = INTER // 128     # 11 tiles over inter dim
# Partial fp8: the first 2 dk-tiles (256 of 2048 contraction dims) of the
# h-phase run as ONE fp8 DoubleRow pass instead of two bf16 passes,
# cutting h-phase PE columns by 1/16. Quantization: x/8 and 8*w in e4m3,
# so the product is unscaled and accumulates raw into the same PSUM
# group as the bf16 dk 2-15 passes. NMAT8=2 applies it to both w1 and w3
# paths (simulated end-to-end rel err 1.65e-2 vs 2e-2 budget; bf16-only
# is 3.7e-3).
NF8 = 2               # dk-tiles in fp8 (one DoubleRow pair)
NMAT8 = 2             # 1 = w1 path only, 2 = w1 and w3
XS8 = 0.125           # x quant scale (x/8)
WS8 = 8.0             # w quant scale (8w)

_PROG = {}
_STATE = {}


def _dedup_ldw(d):
    """Drop redundant PE Ldweights (same weights AP as the currently loaded
    one) from a serialized BIR module. The tile scheduler emits one
    Ldweights per matmul even when consecutive matmuls share the stationary
    operand; each redundant reload forces a pipeline drain + reload
    (~270ns). Waits on a dropped Ldweights that are not dominated by an
    earlier wait in the same engine FIFO are preserved by converting the
    instruction to an EventSemaphore instead of deleting it."""
    import json as _json
    removed = 0
    for fn in d.get("functions", []):
        for blk in fn.get("blocks", []):
            insts = blk.get("instructions", [])
            out = []
            cur_w = None
            waited = {}

            def track(inst):
                for w in (inst.get("sync_info") or {}).get("on_wait", []):
                    if w.get("wait_mode") == "sem-ge-imm":
                        k = (w.get("id"), w.get("ant_name"))
                        v = w.get("wait_value", 0)
                        if v > waited.get(k, -1):
                            waited[k] = v

            for inst in insts:
                if inst.get("engine") != "PE":
                    out.append(inst)
                    continue
                op = inst.get("opcode")
                if op == "Ldweights":
                    sig = _json.dumps(
                        [inst.get("ins"), inst.get("tile_position"),
                         inst.get("tile_size"), inst.get("perf_mode"),
                         inst.get("is_transpose")], sort_keys=True)
                    if sig == cur_w:
                        si = inst.get("sync_info") or {}
                        keep = [w for w in si.get("on_wait", [])
                                if not (w.get("wait_mode") == "sem-ge-imm"
                                        and waited.get(
                                            (w.get("id"), w.get("ant_name")),
                                            -1) >= w.get("wait_value", 0))]
                        ups = si.get("on_update", [])
                        if keep or ups:
                            ev = {"opcode": "EventSemaphore",
                                  "engine": "PE",
                                  "name": inst["name"],
                                  "debug": inst.get("debug"),
                                  "ins": [], "outs": [],
                                  "sync_info": {"on_wait": keep,
                                                "on_update": ups}}
                            track(ev)
                            out.append(ev)
                        removed += 1
                        continue
                    cur_w = sig
                    track(inst)
                    out.append(inst)
                elif op in ("Matmult", "EventSemaphore"):
                    track(inst)
                    out.append(inst)
                else:
                    cur_w = None
                    track(inst)
                    out.append(inst)
            blk["instructions"] = out
    return removed


def _patch_ldw_dedup():
    import concourse.bass_utils as BU
    if getattr(BU, "_ldw_dedup", False):
        return
    orig = BU.bir_verify_and_optimise

    def patched(tmpdir, inp="bir.json", *args, **kw):
        import os
        import json as _json
        try:
            p = os.path.join(str(tmpdir), inp)
            with open(p) as f:
                d = _json.load(f)
            n = _dedup_ldw(d)
            if n:
                with open(p, "w") as f:
                    _json.dump(d, f)
            _STATE["ldw_removed"] = n
        except Exception as e:  # fall back to unmodified BIR
            _STATE["ldw_dedup_error"] = repr(e)
        return orig(tmpdir, inp, *args, **kw)

    BU.bir_verify_and_optimise = patched
    BU._ldw_dedup = True


def build_program():
    if "nc" in _PROG:
        return _PROG["nc"]
    from contextlib import ExitStack
    import concourse.bacc as bacc
    import concourse.mybir as mybir
    import concourse.tile as tile

    _patch_ldw_dedup()

    fp32 = mybir.dt.float32
    bf16 = mybir.dt.bfloat16
    AF = mybir.ActivationFunctionType

    nc = bacc.Bacc("TRN2", target_bir_lowering=False, debug=False,
                   num_devices=NCORES)

    fp8 = mybir.dt.float8e4
    DRow = mybir.MatmulPerfMode.DoubleRow

    # ---- I/O ---- (3 identical 512-token slots per core)
    xts = []
    w13s = []
    w2ts = []
    youts = []
    xq8s = []
    wq8s = []
    for s in range(3):
        xts.append(nc.dram_tensor(f"xt{s}", [128, DI, CAP], bf16,
                                  kind="ExternalInput").ap())
        w13s.append(nc.dram_tensor(f"w13_{s}", [II, 2, 128, DI, 128], bf16,
                                   kind="ExternalInput").ap())
        w2ts.append(nc.dram_tensor(f"w2t_{s}", [II, 128, DIM], bf16,
                                   kind="ExternalInput").ap())
        youts.append(nc.dram_tensor(f"y{s}", [CAP, DIM], bf16,
                                    kind="ExternalOutput").ap())
        xq8s.append(nc.dram_tensor(f"xq8_{s}", [128, NF8, CAP], fp8,
                                   kind="ExternalInput").ap())
        wq8s.append(nc.dram_tensor(f"wq8_{s}", [II, NMAT8, 128, NF8, 128],
                                   fp8, kind="ExternalInput").ap())

    with tile.TileContext(nc) as tc, ExitStack() as ctx:
        xpool = ctx.enter_context(tc.tile_pool(name="xpool", bufs=1))
        wpool = ctx.enter_context(tc.tile_pool(name="wpool", bufs=8))
        w2pool = ctx.enter_context(tc.tile_pool(name="w2pool", bufs=11))
        hpool = ctx.enter_context(tc.tile_pool(name="hpool", bufs=2))
        spool = ctx.enter_context(tc.tile_pool(name="spool", bufs=2))
        ypool = ctx.enter_context(tc.tile_pool(name="ypool", bufs=2))
        psp = ctx.enter_context(tc.tile_pool(name="psp", bufs=4,
                                             space="PSUM"))

        xt_sb = []
        xq8_sb = []
        for s in range(3):
            xsb = xpool.tile([128, DI, CAP], bf16, tag=f"xt{s}",
                             name=f"xt_sb{s}")
            xt_sb.append(xsb)
            xq8_sb.append(xpool.tile([128, NF8, CAP], fp8, tag=f"xq8{s}",
                                     name=f"xq8_sb{s}"))

        def stage_x(s, eng):
            eng.dma_start(out=xq8_sb[s], in_=xq8s[s])
            for g in range(NF8, DI, 4):
                gn = min(4, DI - g)
                eng.dma_start(out=xt_sb[s][:, g:g + gn, :],
                              in_=xts[s][:, g:g + gn, :])

        # Slot order: shared first (its input needs no routing and is
        # staged at startup).
        order = [2, 0, 1]

        # startup: slot-0's x and it0 weights are spread across all
        # three rings in fine chunks, interleaved in the order the PE
        # consumes them (x is the fat item: 2MB; it0-it2 are
        # aggregate-HBM-delivery-bound no matter what).
        s0 = order[0]
        nc.scalar.dma_start(out=xq8_sb[s0], in_=xq8s[s0])

        for idx, s in enumerate(order):
            xsb = xt_sb[s]
            w2sb = [w2pool.tile([128, DIM], bf16, tag="w2", name="w2b")
                    for ib in range(II)]

            # ---- h-phase: W1/W3 stationary, tokens streaming ----
            hT = hpool.tile([128, II, CAP], bf16, tag="hT", name="hT")
            for it in range(II):
                w1b = wpool.tile([128, DI, 128], bf16, tag="w1b", name="w1b")
                w3b = wpool.tile([128, DI, 128], bf16, tag="w3b", name="w3b")
                w1q = wpool.tile([128, NF8, 128], fp8, tag="w1q", name="w1q")
                w3q = wpool.tile([128, NF8, 128], fp8, tag="w3q", name="w3q")
                if it == 0 and idx > 0:
                    # order this slot's h-start after the previous slot's
                    # first y-tile writeback: without this dependency the
                    # scheduler hoists the fp8 matmul ahead of the whole
                    # y-phase in the in-order PE queue, where it blocks
                    # ~8us on a PSUM buffer that only frees mid-y. The
                    # 1-element copy lands before the w1q DMA (WAW), so
                    # the DMA overwrites it -- no data corruption.
                    nc.vector.tensor_copy(w1q[0:1, 0:1, 0:1],
                                          prev_ysb[0:1, 0:1, 0:1])
                if idx == 0 and it == 0:
                    # interleave it0 weight chunks and x slices on both
                    # rings in PE consumption order (dk ascending); the
                    # first matmul fires after just x[0:2]+w1[0:2]+w3[0:2].
                    # Mostly 4-dk chunks: finer slicing shrinks DMA
                    # packets below 1KB and the packet rate caps the
                    # cold-start delivery bandwidth.
                    # late x chunks ride the idle gpsimd ring, which
                    # bursts immediately -- exactly what the cold start
                    # wants (the gated w2 queues behind them).
                    w1d = w13s[s][it, 0]
                    w3d = w13s[s][it, 1]
                    nc.gpsimd.dma_start(out=xsb[:, 6:11, :],
                                        in_=xts[s][:, 6:11, :])
                    nc.gpsimd.dma_start(out=xsb[:, 11:16, :],
                                        in_=xts[s][:, 11:16, :])
                    nc.sync.dma_start(out=w1q, in_=wq8s[s][it, 0])
                    nc.scalar.dma_start(out=w3q, in_=wq8s[s][it, 1])
                    nc.sync.dma_start(out=w1b[:, 2:6, :], in_=w1d[:, 2:6, :])
                    nc.scalar.dma_start(out=xsb[:, 2:6, :],
                                        in_=xts[s][:, 2:6, :])
                    nc.sync.dma_start(out=w1b[:, 6:10, :],
                                      in_=w1d[:, 6:10, :])
                    nc.scalar.dma_start(out=w3b[:, 2:6, :],
                                        in_=w3d[:, 2:6, :])
                    nc.sync.dma_start(out=w1b[:, 10:16, :],
                                      in_=w1d[:, 10:16, :])
                    nc.scalar.dma_start(out=w3b[:, 6:10, :],
                                        in_=w3d[:, 6:10, :])
                    nc.scalar.dma_start(out=w3b[:, 10:16, :],
                                        in_=w3d[:, 10:16, :])
                else:
                    # alternate whole weight-tile pairs between the two
                    # HWDGE rings: halves per-ring bandwidth demand so
                    # the h-phase stream never starves the PE.
                    eng = nc.sync if it % 2 == 0 else nc.scalar
                    eng.dma_start(out=w1q, in_=wq8s[s][it, 0])
                    eng.dma_start(out=w3q, in_=wq8s[s][it, 1])
                    eng.dma_start(out=w1b[:, NF8:, :],
                                  in_=w13s[s][it, 0][:, NF8:, :])
                    eng.dma_start(out=w3b[:, NF8:, :],
                                  in_=w13s[s][it, 1][:, NF8:, :])
                # w2^T prefetch on the gpsimd SWDGE ring. Slot 0: gated
                # by a WAW dependency on a tiny DVE write into w2sb[0]
                # that itself depends on it3's s1 -- without it the idle
                # gpsimd ring bursts all 5.6MB at t~5us and starves the
                # cold-start x/w13 streams of HBM bandwidth (the FIFO
                # serializes the remaining tiles behind the gated one).
                # Slots 1-2: naturally gated by w2pool buffer release
                # from the previous slot's y-phase.
                if it == 4:
                    if idx == 0:
                        # slot 0: first 3 w2 tiles ride the sync ring
                        # (issued at it6, FIFO-deferred past the cold
                        # window) so the y-phase's first reads never
                        # race the gated gpsimd burst carrying the rest.
                        nc.vector.tensor_copy(w2sb[3][0:1, 0:1],
                                              last_s1[0:1, 0:1])
                        for ib in range(3, II):
                            nc.gpsimd.dma_start(out=w2sb[ib],
                                                in_=w2ts[s][ib])
                    else:
                        for ib in range(II):
                            nc.gpsimd.dma_start(out=w2sb[ib],
                                                in_=w2ts[s][ib])
                if idx == 0 and it == 6:
                    for ib in range(3):
                        nc.sync.dma_start(out=w2sb[ib], in_=w2ts[s][ib])
                if it == II - 1 and idx < 2:
                    stage_x(order[idx + 1], nc.sync)
                ph1 = psp.tile([128, 512], fp32, tag="ps", name="ph1")
                ph3 = psp.tile([128, 512], fp32, tag="ps", name="ph3")
                # dk 0..NF8-1 as one fp8 DoubleRow pass (unscaled
                # product: x/8 quantized against 8*w), then bf16 passes
                # accumulate into the same PSUM group.
                nc.tensor.matmul(ph1, lhsT=w1q, rhs=xq8_sb[s],
                                 start=True, stop=False, perf_mode=DRow,
                                 skip_group_check=True)
                nc.tensor.matmul(ph3, lhsT=w3q, rhs=xq8_sb[s],
                                 start=True, stop=False, perf_mode=DRow,
                                 skip_group_check=True)
                for dk in range(NF8, DI):
                    sp = dk == DI - 1
                    nc.tensor.matmul(ph1, lhsT=w1b[:, dk, :],
                                     rhs=xsb[:, dk, :],
                                     start=False, stop=sp,
                                     skip_group_check=True)
                    nc.tensor.matmul(ph3, lhsT=w3b[:, dk, :],
                                     rhs=xsb[:, dk, :],
                                     start=False, stop=sp,
                                     skip_group_check=True)
                s1 = spool.tile([128, CAP], fp32, tag="s1", name="s1")
                nc.scalar.activation(s1, ph1, AF.Silu)
                nc.vector.tensor_mul(hT[:, it, :], s1, ph3)
                last_s1 = s1

            # ---- y-phase: h tiles stationary, W2^T streaming ----
            for tt in range(CAP // 128):
                t0 = tt * 128
                ya = psp.tile([128, 2, 512], fp32, tag="ps", name="ya")
                yb = psp.tile([128, 2, 512], fp32, tag="ps", name="yb")
                for ib in range(II):
                    st = ib == 0
                    sp = ib == II - 1
                    lhs = hT[:, ib, t0:t0 + 128]
                    nc.tensor.matmul(ya[:, 0, :], lhsT=lhs,
                                     rhs=w2sb[ib][:, 0:512],
                                     start=st, stop=sp)
                    nc.tensor.matmul(ya[:, 1, :], lhsT=lhs,
                                     rhs=w2sb[ib][:, 512:1024],
                                     start=st, stop=sp)
                    nc.tensor.matmul(yb[:, 0, :], lhsT=lhs,
                                     rhs=w2sb[ib][:, 1024:1536],
                                     start=st, stop=sp)
                    nc.tensor.matmul(yb[:, 1, :], lhsT=lhs,
                                     rhs=w2sb[ib][:, 1536:2048],
                                     start=st, stop=sp)
                ysb = ypool.tile([128, 4, 512], bf16, tag="ysb", name="ysb")
                if tt == 1:
                    # anchor one y-tile deeper: by then the scalar-FIFO
                    # triggers gated on PE matmul counts have cleared, so
                    # the next slot's h-start finds its PSUM/silu deps
                    # pre-satisfied (tt==0 left a ~2.2us residual stall).
                    prev_ysb = ysb
                last = idx == 2 and tt == CAP // 128 - 1
                nc.scalar.copy(ysb[:, 0, :], ya[:, 0, :])
                nc.vector.tensor_copy(ysb[:, 1, :], ya[:, 1, :])
                nc.scalar.dma_start(
                    out=youts[s][t0:t0 + 128, 0:1024],
                    in_=ysb[:, 0:2].rearrange("p a b -> p (a b)"))
                nc.scalar.copy(ysb[:, 2, :], yb[:, 0, :])
                nc.vector.tensor_copy(ysb[:, 3, :], yb[:, 1, :])
                if last:
                    # split the very last writeback across two rings so
                    # the kernel-exit drain waits on half-size DMAs
                    nc.scalar.dma_start(out=youts[s][t0:t0 + 128, 1024:1536],
                                        in_=ysb[:, 2, :])
                    nc.sync.dma_start(out=youts[s][t0:t0 + 128, 1536:2048],
                                      in_=ysb[:, 3, :])
                else:
                    nc.scalar.dma_start(
                        out=youts[s][t0:t0 + 128, 1024:2048],
                        in_=ysb[:, 2:4].rearrange("p a b -> p (a b)"))

    nc.compile()
    _PROG["nc"] = nc
    return nc


def _route(xf, gate_w):
    # fp64 gate: softmax over routed experts, top-2 (matches fp32 ref
    # ordering -- min top2/top3 logit gap >> fp64 matmul error)
    logits = xf.astype(np.float64) @ np.asarray(gate_w, np.float64).T
    p = np.exp(logits - logits.max(-1, keepdims=True))
    p /= p.sum(-1, keepdims=True)
    idx = np.argsort(-p, axis=-1)[:, :TOPK]          # [T, 2]
    wts = np.take_along_axis(p, idx, axis=-1)        # [T, 2]
    return idx.astype(np.int64), wts.astype(np.float32)


def _make_slots(idx, wts):
    """Assign (expert, token-list, weight-list) to 16 uniform slots of
    CAP=512 (2 per core). Each expert's first <=CAP tokens fill one slot;
    the largest overflow chunk fills the one spare slot; all remaining
    overflow goes to the host numpy fallback. Returns (slots0, slots1,
    leftovers); each slot is (expert, tokens, weights)."""
    ntok = idx.shape[0]
    per_e_tok = [[] for _ in range(NE)]
    per_e_w = [[] for _ in range(NE)]
    flat_t = np.repeat(np.arange(ntok), TOPK)
    flat_e = idx.reshape(-1)
    flat_w = wts.reshape(-1)
    order = np.argsort(flat_e, kind="stable")
    for e, t, w in zip(flat_e[order], flat_t[order], flat_w[order]):
        per_e_tok[e].append(t)
        per_e_w[e].append(w)

    slots = []
    overflow = []
    for e in range(NE):
        toks = np.array(per_e_tok[e], np.int64)
        ws = np.array(per_e_w[e], np.float32)
        slots.append((e, toks[:CAP], ws[:CAP]))
        for o in range(CAP, len(toks), CAP):
            overflow.append((e, toks[o:o + CAP], ws[o:o + CAP]))
    overflow.sort(key=lambda x: -len(x[1]))
    nspare = 16 - len(slots)
    spare, leftovers = overflow[:nspare], overflow[nspare:]
    slots.extend(spare)
    while len(slots) < 16:
        slots.append((0, np.zeros(0, np.int64), np.zeros(0, np.float32)))
    # balanced: largest with smallest per core (all caps equal anyway)
    slots.sort(key=lambda x: -len(x[1]))
    slots0 = slots[:8]
    slots1 = slots[8:][::-1]
    return slots0, slots1, leftovers


def _pack13(w1e, w3e, bf):
    # [INTER, DIM] x2 -> [II, 2, 128, DI, 128] stationary lhsT tiles
    out = np.empty((II, 2, 128, DI, 128), bf)
    for m, w in ((0, w1e), (1, w3e)):
        out[:, m] = np.asarray(w, np.float32).reshape(
            II, 128, DI, 128).transpose(0, 3, 2, 1).astype(bf)
    return np.ascontiguousarray(out)


def _pack2(w2e, bf):
    # [DIM, INTER] -> [II, 128, DIM] moving w2^T tiles
    return np.ascontiguousarray(
        np.asarray(w2e, np.float32).T.reshape(II, 128, DIM)).astype(bf)


def _packx(xf_rows, cap, bf):
    # [n, DIM] fp32 -> [128, DI, cap] bf16 (zero-padded)
    n = xf_rows.shape[0]
    out = np.zeros((128, DI, cap), bf)
    if n:
        out[:, :, :n] = xf_rows.T.reshape(DI, 128, n).transpose(1, 0, 2).astype(bf)
    return out


def _packx8(xf_rows, cap, f8):
    # [n, DIM] fp32 -> [128, NF8, cap] e4m3 of x*XS8 (first NF8 dk-tiles)
    n = xf_rows.shape[0]
    out = np.zeros((128, NF8, cap), f8)
    if n:
        blk = np.clip(xf_rows[:, :NF8 * 128].T * XS8, -240, 240)
        out[:, :, :n] = blk.reshape(NF8, 128, n).transpose(1, 0, 2).astype(f8)
    return out


def _pack13q8(w1e, w3e, f8):
    # first NF8 dk-tiles of w1/w3 * WS8 -> [II, NMAT8, 128, NF8, 128]
    out = np.empty((II, NMAT8, 128, NF8, 128), f8)
    for m, w in ((0, w1e), (1, w3e))[:NMAT8]:
        ww = np.clip(np.asarray(w, np.float32)[:, :NF8 * 128] * WS8,
                     -240, 240)
        out[:, m] = ww.reshape(II, 128, NF8, 128).transpose(
            0, 3, 2, 1).astype(f8)
    return np.ascontiguousarray(out)


def prep_in_maps(x, gate_w, w1, w2, w3, sw1, sw2, sw3):
    bf = ml_dtypes.bfloat16
    f8 = ml_dtypes.float8_e4m3
    xf = np.ascontiguousarray(np.asarray(x, np.float32).reshape(-1, DIM))
    ntok = xf.shape[0]
    assert ntok == T and xf.shape[1] == DIM

    idx, wts = _route(xf, gate_w)
    slots0, slots1, leftovers = _make_slots(idx, wts)
    _STATE["slots0"] = slots0
    _STATE["slots1"] = slots1
    _STATE["leftovers"] = leftovers
    _STATE["inputs"] = (xf, w1, w2, w3)

    pack13_cache = {}
    pack2_cache = {}
    packq8_cache = {}

    def get13(e):
        if e not in pack13_cache:
            pack13_cache[e] = _pack13(w1[e], w3[e], bf)
        return pack13_cache[e]

    def get2(e):
        if e not in pack2_cache:
            pack2_cache[e] = _pack2(w2[e], bf)
        return pack2_cache[e]

    def getq8(e):
        if e not in packq8_cache:
            packq8_cache[e] = _pack13q8(w1[e], w3[e], f8)
        return packq8_cache[e]

    sh13 = _pack13(sw1, sw3, bf)
    sh2 = _pack2(sw2, bf)
    shq8 = _pack13q8(sw1, sw3, f8)

    in_maps = []
    for c in range(NCORES):
        e0, t0, _ = slots0[c]
        e1, t1, _ = slots1[c]
        in_maps.append({
            "xt0": _packx(xf[t0], CAP, bf),
            "xt1": _packx(xf[t1], CAP, bf),
            "xt2": _packx(xf[c * CAP:(c + 1) * CAP], CAP, bf),
            "xq8_0": _packx8(xf[t0], CAP, f8),
            "xq8_1": _packx8(xf[t1], CAP, f8),
            "xq8_2": _packx8(xf[c * CAP:(c + 1) * CAP], CAP, f8),
            "w13_0": get13(e0), "w2t_0": get2(e0), "wq8_0": getq8(e0),
            "w13_1": get13(e1), "w2t_1": get2(e1), "wq8_1": getq8(e1),
            "w13_2": sh13, "w2t_2": sh2, "wq8_2": shq8,
        })
    return in_maps


def assemble(results, out_shape):
    y = np.zeros((T, DIM), np.float32)
    slots0, slots1 = _STATE["slots0"], _STATE["slots1"]
    for c in range(NCORES):
        r = results[c]
        for slots, key in ((slots0, "y0"), (slots1, "y1")):
            _, toks, ws = slots[c]
            n = len(toks)
            if n:
                blk = np.asarray(r[key])[:n].astype(np.float32)
                np.add.at(y, toks, blk * ws[:, None])
        y[c * CAP:(c + 1) * CAP] += np.asarray(r["y2"]).astype(np.float32)
    # numpy fallback for any tokens that did not fit the static capacities
    leftovers = _STATE["leftovers"]
    if leftovers:
        xf, w1, w2, w3 = _STATE["inputs"]
        for e, toks, ws in leftovers:
            if len(toks) == 0:
                continue
            xe = xf[toks]
            h1 = xe @ np.asarray(w1[e], np.float32).T
            h3 = xe @ np.asarray(w3[e], np.float32).T
            h = (h1 / (1 + np.exp(-h1))) * h3
            y[toks] += (h @ np.asarray(w2[e], np.float32).T) * ws[:, None]
    return y.reshape(out_shape)


def run_on_hw(in_maps, trace=False, tmpdir=None):
    from concourse.bass_utils import run_bass_kernel_spmd
    nc = build_program()
    return run_bass_kernel_spmd(nc, in_maps, list(range(NCORES)),
                                trace=trace, tmpdir=tmpdir)


def kernel(x, gate_w, w1, w2, w3, sw1, sw2, sw3):
    in_maps = prep_in_maps(x, gate_w, w1, w2, w3, sw1, sw2, sw3)
    br = run_on_hw(in_maps)
    return assemble(br.results, np.asarray(x).shape)
